# revision 8
# baseline (speedup 1.0000x reference)
"""CosineSimCodebook (VQ) Trainium2 kernel, 8-core SPMD.

Strategy (per sharding hint): data-parallel shard the 16384 flattened
tokens across 8 cores (2048 each), replicate the [8192, 256] codebook.
Each core:
  1. normalizes the codebook (en) on device, builds bf16 hi/lo splits of
     x^T and en^T (PE transposes), computes dist = x @ en.T as three bf16
     matmuls (hi*hi + hi*lo + lo*hi) accumulated in fp32 PSUM --
     fp32-accurate, 3x faster than native fp32 matmul (4 cyc/row).
     Row-scaling of x does not change the argmax, so x is not normalized
     for the dist matmul.
  2. argmax per token via DVE max8 + max_index.
  3. scatter phase: for each 1024-wide codebook shard, rebuild onehot
     tiles from the indices (iota is_equal) and accumulate
     embed_sum[k,0:256] and bins[k]=col 256 with bf16 matmuls
     (onehot^T @ [fn | 1]) in PSUM across all 16 token tiles.
  4. ReduceScatter the [8192, 258] partial sums; each core EMA-updates
     its own 1024-row codebook shard (l2norm(embed_sum) == the
     reference's l2norm(embed_sum/bins) since l2norm is scale-invariant;
     bins only matter via the bins==0 mask).
Host: concatenates index/embed_new shards; quantize = embed[ind] gather.
"""

import sys

sys.path.insert(0, "/opt/trn_rl_repo")

import numpy as np

N_CORES = 8
TOK = 2048          # tokens per core
TT = TOK // 128     # 16 token tiles
K = 8192
D = 256
KC = K // 512       # 16 dist chunks of 512
NSH = 8             # codebook shards for scatter phase
KS = K // NSH       # 1024
DECAY = 0.8

_cache = {}


def _build(profile_variant=False):
    import concourse.bacc as bacc
    import concourse.mybir as mybir
    from concourse import tile

    f32 = mybir.dt.float32
    bf16 = mybir.dt.bfloat16
    u32 = mybir.dt.uint32

    nc = bacc.Bacc()
    x_in = nc.dram_tensor("x_sh", [TOK, D], f32, kind="ExternalInput")
    e_in = nc.dram_tensor("embed", [K, D], f32, kind="ExternalInput")
    esh_in = nc.dram_tensor("embed_sh", [KS, D], f32, kind="ExternalInput")
    ind_out = nc.dram_tensor("ind_out", [128, TT], u32, kind="ExternalOutput")
    new_out = nc.dram_tensor("new_out", [KS, D], f32, kind="ExternalOutput")

    with tile.TileContext(nc) as tc:
        with (
            tc.tile_pool(name="cst", bufs=1) as cst,
            tc.tile_pool(name="big", bufs=1) as big,
            tc.tile_pool(name="str", bufs=10) as str_,
            tc.tile_pool(name="dst", bufs=2) as dstp,
            tc.tile_pool(name="sml", bufs=4) as sml,
            tc.tile_pool(name="ps", bufs=8, space="PSUM") as ps,
            tc.tile_pool(name="dram", bufs=1, space="DRAM") as dram,
        ):
            cc_in = dram.tile([K, 258], bf16, tag="cc_in")
            cc_out = dram.tile([KS, 258], bf16, tag="cc_out")
            ident = cst.tile([128, 128], bf16, tag="ident")
            nc.gpsimd.memset(ident[:], 0.0)
            nc.gpsimd.iota(
                ident[:],
                pattern=[[1, 128]],
                base=0,
                channel_multiplier=0,
                allow_small_or_imprecise_dtypes=True,
            )
            idp = cst.tile([128, 1], f32, tag="idp")
            nc.gpsimd.iota(
                idp[:], pattern=[[0, 1]], base=0, channel_multiplier=1,
                allow_small_or_imprecise_dtypes=True,
            )
            # ident = (iota_row == partition_idx)
            nc.vector.tensor_scalar(
                ident[:], ident[:], idp[:], None, op0=mybir.AluOpType.is_equal
            )

            iot = cst.tile([128, KS], f32, tag="iot")
            nc.gpsimd.iota(
                iot[:], pattern=[[1, KS]], base=0, channel_multiplier=0,
                allow_small_or_imprecise_dtypes=True,
            )

            # ---- codebook: normalize + bf16 hi/lo transposed copies ----
            ehT = big.tile([128, 2 * K], bf16, tag="ehT")   # [d0|d1] halves
            elT = big.tile([128, 2 * K], bf16, tag="elT")
            for i in range(K // 128):
                et = str_.tile([128, D], f32, tag="w")
                nc.sync.dma_start(et[:], e_in[i * 128 : (i + 1) * 128, :])
                ss = sml.tile([128, 1], f32, tag="ss")
                zz = str_.tile([128, D], f32, tag="w")
                nc.scalar.activation(
                    zz[:], et[:], mybir.ActivationFunctionType.Square,
                    accum_out=ss[:],
                )
                sq = sml.tile([128, 1], f32, tag="sq")
                nc.scalar.activation(sq[:], ss[:], mybir.ActivationFunctionType.Sqrt)
                rr = sml.tile([128, 1], f32, tag="rr")
                nc.vector.reciprocal(rr[:], sq[:])
                en = str_.tile([128, D], f32, tag="w")
                nc.scalar.activation(
                    en[:], et[:], mybir.ActivationFunctionType.Copy, scale=rr[:]
                )
                enh = str_.tile([128, D], bf16, tag="wb")
                nc.vector.tensor_copy(enh[:], en[:])
                enl = str_.tile([128, D], bf16, tag="wb")
                nc.vector.tensor_tensor(
                    enl[:], en[:], enh[:], op=mybir.AluOpType.subtract
                )
                for dh in range(2):
                    for src, dst in ((enh, ehT), (enl, elT)):
                        pt = ps.tile([128, 128], bf16, tag="ps")
                        nc.tensor.transpose(
                            pt[:], src[:, dh * 128 : (dh + 1) * 128], ident[:]
                        )
                        nc.vector.tensor_copy(
                            dst[:, dh * K + i * 128 : dh * K + (i + 1) * 128], pt[:]
                        )

            # ---- x: fn (normalized, bf16, with ones col) + hi/lo x^T ----
            xhT = big.tile([128, 2 * TOK], bf16, tag="xhT")
            xlT = big.tile([128, 2 * TOK], bf16, tag="xlT")
            fnx = big.tile([128, TT * 258], bf16, tag="fnx")
            nc.gpsimd.memset(fnx[:], 0.0)
            for t in range(TT):
                xt = str_.tile([128, D], f32, tag="w")
                nc.sync.dma_start(xt[:], x_in[t * 128 : (t + 1) * 128, :])
                ss = sml.tile([128, 1], f32, tag="ss")
                zz = str_.tile([128, D], f32, tag="w")
                nc.scalar.activation(
                    zz[:], xt[:], mybir.ActivationFunctionType.Square,
                    accum_out=ss[:],
                )
                sq = sml.tile([128, 1], f32, tag="sq")
                nc.scalar.activation(sq[:], ss[:], mybir.ActivationFunctionType.Sqrt)
                rr = sml.tile([128, 1], f32, tag="rr")
                nc.vector.reciprocal(rr[:], sq[:])
                nc.scalar.activation(
                    fnx[:, t * 258 : t * 258 + 256], xt[:],
                    mybir.ActivationFunctionType.Copy, scale=rr[:],
                )
                nc.vector.memset(fnx[:, t * 258 + 256 : t * 258 + 257], 1.0)
                xh = str_.tile([128, D], bf16, tag="wb")
                nc.vector.tensor_copy(xh[:], xt[:])
                xl = str_.tile([128, D], bf16, tag="wb")
                nc.vector.tensor_tensor(
                    xl[:], xt[:], xh[:], op=mybir.AluOpType.subtract
                )
                for dh in range(2):
                    for src, dst in ((xh, xhT), (xl, xlT)):
                        pt = ps.tile([128, 128], bf16, tag="ps")
                        nc.tensor.transpose(
                            pt[:], src[:, dh * 128 : (dh + 1) * 128], ident[:]
                        )
                        nc.vector.tensor_copy(
                            dst[:, dh * TOK + t * 128 : dh * TOK + (t + 1) * 128],
                            pt[:],
                        )

            # ---- phase 1: dist + argmax per token tile ----
            idxf = cst.tile([128, TT], f32, tag="idxf")
            idxu = cst.tile([128, TT], u32, tag="idxu")
            terms = ((xhT, ehT), (xhT, elT), (xlT, ehT))
            for t in range(TT):
                dsb = dstp.tile([128, K], f32, tag="dsb")
                for h in range(2):
                    pts = [ps.tile([128, 512], f32, tag="ps", name=f"pts{c}") for c in range(8)]
                    for ti, (xw, ew) in enumerate(terms):
                        for dh in range(2):
                            lhs = xw[:, dh * TOK + t * 128 : dh * TOK + (t + 1) * 128]
                            for c in range(8):
                                kc = h * 8 + c
                                nc.tensor.matmul(
                                    pts[c][:],
                                    lhs,
                                    ew[:, dh * K + kc * 512 : dh * K + (kc + 1) * 512],
                                    start=(ti == 0 and dh == 0),
                                    stop=(ti == 2 and dh == 1),
                                )
                    for c in range(8):
                        kc = h * 8 + c
                        nc.scalar.copy(dsb[:, kc * 512 : (kc + 1) * 512], pts[c][:])
                m8 = sml.tile([128, 8], f32, tag="m8")
                nc.vector.max(m8[:], dsb[:])
                i8 = sml.tile([128, 8], u32, tag="i8")
                nc.vector.max_index(i8[:], m8[:], dsb[:])
                nc.vector.tensor_copy(idxu[:, t : t + 1], i8[:, 0:1])
                nc.vector.tensor_copy(idxf[:, t : t + 1], i8[:, 0:1])
            nc.sync.dma_start(ind_out[:], idxu[:])

            # ---- phase 2: scatter (embed_sum | bins) per codebook shard ----
            for s in range(NSH):
                accs = [ps.tile([128, 258], f32, tag="ps", name=f"acc{k}") for k in range(8)]
                idxs = sml.tile([128, TT], f32, tag="idxs")
                nc.vector.tensor_scalar(
                    idxs[:], idxf[:], float(s * KS), None,
                    op0=mybir.AluOpType.subtract,
                )
                for t in range(TT):
                    oh = sml.tile([128, KS], bf16, tag="oh")
                    nc.vector.tensor_scalar(
                        oh[:],
                        iot[:],
                        idxs[:, t : t + 1],
                        None,
                        op0=mybir.AluOpType.is_equal,
                    )
                    rhs = fnx[:, t * 258 : (t + 1) * 258]
                    for k in range(8):
                        nc.tensor.matmul(
                            accs[k][:],
                            oh[:, k * 128 : (k + 1) * 128],
                            rhs,
                            start=(t == 0),
                            stop=(t == TT - 1),
                        )
                for k in range(8):
                    ev = str_.tile([128, 258], bf16, tag="evb")
                    nc.scalar.copy(ev[:], accs[k][:])
                    nc.sync.dma_start(
                        cc_in[(s * 8 + k) * 128 : (s * 8 + k + 1) * 128, :], ev[:]
                    )

            if profile_variant:
                cc_out = cc_in  # skip collective for cost-model profiling
            else:
                nc.gpsimd.collective_compute(
                    "ReduceScatter",
                    mybir.AluOpType.add,
                    replica_groups=[list(range(N_CORES))],
                    ins=[cc_in[:, :].opt()],
                    outs=[cc_out[:, :].opt()],
                )

            # ---- EMA update of this core's codebook shard ----
            for j in range(KS // 128):
                rs = str_.tile([128, 258], bf16, tag="evb")
                nc.sync.dma_start(rs[:], cc_out[j * 128 : (j + 1) * 128, :])
                ss = sml.tile([128, 1], f32, tag="ss")
                zz = str_.tile([128, D], f32, tag="w")
                nc.scalar.activation(
                    zz[:], rs[:, 0:256], mybir.ActivationFunctionType.Square,
                    accum_out=ss[:],
                )
                sq = sml.tile([128, 1], f32, tag="sq")
                nc.scalar.activation(sq[:], ss[:], mybir.ActivationFunctionType.Sqrt)
                sm = sml.tile([128, 1], f32, tag="sm")
                nc.vector.tensor_scalar(
                    sm[:], sq[:], 1e-12, None, op0=mybir.AluOpType.max
                )
                rr = sml.tile([128, 1], f32, tag="rr")
                nc.vector.reciprocal(rr[:], sm[:])
                r02 = sml.tile([128, 1], f32, tag="r02")
                nc.vector.tensor_scalar(
                    r02[:], rr[:], 1.0 - DECAY, None, op0=mybir.AluOpType.mult
                )
                zm = sml.tile([128, 1], f32, tag="zm")
                nc.vector.tensor_scalar(
                    zm[:], rs[:, 256:257], 0.0, None, op0=mybir.AluOpType.is_equal
                )
                l2n = str_.tile([128, D], f32, tag="w")
                nc.scalar.activation(
                    l2n[:], rs[:, 0:256], mybir.ActivationFunctionType.Copy,
                    scale=r02[:],
                )
                # en rows (for empty bins): recompute from embed shard rows
                et = str_.tile([128, D], f32, tag="w")
                nc.gpsimd.dma_start(et[:], esh_in[j * 128 : (j + 1) * 128, :])
                ss2 = sml.tile([128, 1], f32, tag="ss2")
                zz2 = str_.tile([128, D], f32, tag="w")
                nc.scalar.activation(
                    zz2[:], et[:], mybir.ActivationFunctionType.Square,
                    accum_out=ss2[:],
                )
                sq2 = sml.tile([128, 1], f32, tag="sq2")
                nc.scalar.activation(sq2[:], ss2[:], mybir.ActivationFunctionType.Sqrt)
                rr2 = sml.tile([128, 1], f32, tag="rr2")
                nc.vector.reciprocal(rr2[:], sq2[:])
                # combined scale: (1/||e||) * zmask * 0.2
                cs = sml.tile([128, 1], f32, tag="cs")
                nc.vector.tensor_tensor(
                    cs[:], rr2[:], zm[:], op=mybir.AluOpType.mult
                )
                nc.vector.tensor_scalar(
                    cs[:], cs[:], 1.0 - DECAY, None, op0=mybir.AluOpType.mult
                )
                enz = str_.tile([128, D], f32, tag="w")
                nc.scalar.activation(
                    enz[:], et[:], mybir.ActivationFunctionType.Copy, scale=cs[:]
                )
                upd = str_.tile([128, D], f32, tag="w")
                nc.vector.tensor_tensor(
                    upd[:], l2n[:], enz[:], op=mybir.AluOpType.add
                )
                out = str_.tile([128, D], f32, tag="w")
                nc.vector.scalar_tensor_tensor(
                    out[:], et[:], DECAY, upd[:],
                    op0=mybir.AluOpType.mult, op1=mybir.AluOpType.add,
                )
                nc.sync.dma_start(new_out[j * 128 : (j + 1) * 128, :], out[:])

    nc.compile()
    return nc


def _get_nc():
    if "nc" not in _cache:
        _cache["nc"] = _build()
    return _cache["nc"]


def kernel(x: np.ndarray, embed: np.ndarray):
    from concourse.bass_utils import run_bass_kernel_spmd

    nc = _get_nc()
    x = np.ascontiguousarray(x, dtype=np.float32)
    embed = np.ascontiguousarray(embed, dtype=np.float32)
    flat = x.reshape(-1, D)
    in_maps = [
        {
            "x_sh": flat[c * TOK : (c + 1) * TOK],
            "embed": embed,
            "embed_sh": embed[c * KS : (c + 1) * KS],
        }
        for c in range(N_CORES)
    ]
    res = run_bass_kernel_spmd(nc, in_maps, list(range(N_CORES)))
    ind = np.concatenate(
        [res.results[c]["ind_out"].T.reshape(-1) for c in range(N_CORES)]
    ).astype(np.int32)
    embed_new = np.concatenate(
        [res.results[c]["new_out"] for c in range(N_CORES)], axis=0
    )
    embed_ind = ind.reshape(x.shape[:-1])
    quantize = embed[embed_ind]
    return quantize, embed_ind, embed_new
